# revision 2
# baseline (speedup 1.0000x reference)
"""BCMSE loss kernel v3 for 8 Trainium2 NeuronCores.

Host layout as baseline: batch split across 8 cores, columns permuted to
scalar(2)|vec(3)|angle(4), fp16, pre-tiled [n_tiles*P, 9*q].

Engine assignment (measured rates, us per elem/row-unit per core:
DVE ts2-noaccum 1.17, DVE tt 2.25, ACT 3.69, GP tt 7.86, PE ~73ns/mm):

  DVE : y=(o-.5)+M (fp16 round -> floor+M), fl=y-M (in-place),
        d0a=o-t, dn=fl-d0a (in-place over d0a),
        r1=relu(dn-.5), m2=min(dn+.5,0)  (ts2, no accum!),
        w=[v<0] (ts2), a=v-t
  GP  : wn_b=w_b*n (in-place over w), n2=sq0+sq1+sq2
  ACT : ext=sum|fl| (Abs accum), vsq, nrm=sqrt(n2) (+sum), sum dn^2 (Square)
  PE  : sum r1 / sum m2 (ones-moving matmuls into [P,1] psum),
        p0 Grams (oo/ot/tt diag), p2 Grams (aa/aw/ww diag), PSUM-accumulated
  host: p1 = sum dn^2 - 2*(R1 - M2 + .5 N4) + N4,
        p0 = trOO - 2 trOT + trTT, p2 = trAA + 2 trAW + trWW, final combine.

Identities: err^2 = min(d^2,(|d|-1)^2) = d^2 - (2 max(|d|,.5) - 1);
max(|d|,.5) = relu(d-.5) + relu(-d-.5) + .5;  remainder(v,n) = v + n*[v<0].
"""
import numpy as np

import concourse.bacc as bacc
import concourse.bass as bass
import concourse.mybir as mybir
from concourse.tile import TileContext
from concourse.bass_utils import run_bass_kernel_spmd

N_CORES = 8
BATCH = 4194304
SHARD = BATCH // N_CORES
P = 128
Q = 512
TILE_ROWS = P * Q
N_TILES = SHARD // TILE_ROWS      # 4
PERM = [0, 3, 6, 7, 8, 1, 2, 4, 5]
MAGIC_H = 1536.0
HALF = True
CONSTANT_WEIGHT = 10.0

f16 = mybir.dt.float16
f32 = mybir.dt.float32
AF = mybir.ActivationFunctionType
OP = mybir.AluOpType

_cache = {}

# gsb: [OO, OT, TT, AA, AW, WW] then DN gram, then R1 col
NG = 7


def _build(shard, q, n_tiles, reps=1, mode='full', half=True, skip=(),
           iob=None, scrb=None, packed=False):
    if iob is None:
        iob = 3 if q <= 512 else 2
    if scrb is None:
        scrb = 3
    assert half
    dt = f16
    M = MAGIC_H
    nc = bacc.Bacc("TRN2", target_bir_lowering=False)
    o_d = nc.dram_tensor("o", [n_tiles * P, 9 * q], dt, kind="ExternalInput")
    t_d = nc.dram_tensor("t", [n_tiles * P, 9 * q], dt, kind="ExternalInput")
    out_d = nc.dram_tensor("partials", [P, 8], f32, kind="ExternalOutput")
    gram_d = nc.dram_tensor("gram", [P, NG * P + 1], f32, kind="ExternalOutput")

    with TileContext(nc) as tc:
        with (
            tc.tile_pool(name="io", bufs=iob) as io,
            tc.tile_pool(name="scr", bufs=scrb) as scr,
            tc.tile_pool(name="acc", bufs=1) as acc,
            tc.tile_pool(name="ps", bufs=1, space="PSUM") as psp,
        ):
            ones1 = acc.tile([P, 1], dt, tag="ones1")
            nc.vector.memset(ones1[:], 1.0)
            negh = acc.tile([P, 1], f32, tag="negh")
            nc.vector.memset(negh[:], -0.5)
            s_ext = acc.tile([P, n_tiles], f32, tag="s_ext")
            s_r2 = acc.tile([P, n_tiles], f32, tag="s_r2")
            s_nrm = acc.tile([P, n_tiles], f32, tag="s_nrm")
            accs = [s_ext, s_r2, s_nrm]
            if mode == 'dma':
                for s in accs:
                    nc.vector.memset(s[:], 0.0)

            pOO = psp.tile([P, P], f32, tag="pOO")
            pOT = psp.tile([P, P], f32, tag="pOT")
            pTT = psp.tile([P, P], f32, tag="pTT")
            pAA = psp.tile([P, P], f32, tag="pAA")
            pAW = psp.tile([P, P], f32, tag="pAW")
            pWW = psp.tile([P, P], f32, tag="pWW")
            pR1 = psp.tile([P, 1], f32, tag="pR1")
            pDN = psp.tile([P, P], f32, tag="pDN")

            from contextlib import nullcontext
            loop = tc.For_i(0, reps, 1) if reps > 1 else nullcontext()
            with loop:
              for i in range(n_tiles):
                ot = io.tile([P, 9 * q], dt, tag="ot")
                tt = io.tile([P, 9 * q], dt, tag="tt")
                wid = 9 * q // 8 if mode == 'nodma' else 9 * q
                nc.sync.dma_start(out=ot[:, 0:wid], in_=o_d[i * P:(i + 1) * P, 0:wid])
                nc.sync.dma_start(out=tt[:, 0:wid], in_=t_d[i * P:(i + 1) * P, 0:wid])
                if mode == 'dma':
                    continue
                o_sc, t_sc = ot[:, 0:2 * q], tt[:, 0:2 * q]
                o_v, t_v = ot[:, 2 * q:5 * q], tt[:, 2 * q:5 * q]
                o_a, t_a = ot[:, 5 * q:9 * q], tt[:, 5 * q:9 * q]
                first, last = (i == 0), (i == n_tiles - 1)

                # ---------- angle (DVE chain) ----------
                y = scr.tile([P, 4 * q], dt, tag="y")
                nc.vector.tensor_scalar(out=y[:], in0=o_a, scalar1=0.5,
                                        scalar2=M, op0=OP.subtract, op1=OP.add)
                # fl = y - M, in place
                nc.vector.tensor_scalar(out=y[:], in0=y[:], scalar1=M,
                                        scalar2=None, op0=OP.subtract)
                if packed:
                    # d0a/dn live in the t-angle io region (its only consumer)
                    nc.vector.tensor_tensor(out=t_a, in0=o_a, in1=t_a,
                                            op=OP.subtract)
                    nc.vector.tensor_tensor(out=t_a, in0=y[:], in1=t_a,
                                            op=OP.subtract)
                    dn = t_a
                    # ext after dn: Abs in place over y
                    nc.scalar.activation(out=y[:], in_=y[:], func=AF.Abs,
                                         accum_out=s_ext[:, i:i + 1])
                else:
                    jA = scr.tile([P, 4 * q], mybir.dt.float8e4, tag="jA")
                    nc.scalar.activation(out=jA[:], in_=y[:], func=AF.Abs,
                                         accum_out=s_ext[:, i:i + 1])
                    d0a = scr.tile([P, 4 * q], dt, tag="d0a")
                    nc.vector.tensor_tensor(out=d0a[:], in0=o_a, in1=t_a,
                                            op=OP.subtract)
                    nc.vector.tensor_tensor(out=d0a[:], in0=y[:], in1=d0a[:],
                                            op=OP.subtract)
                    dn = d0a
                r1 = scr.tile([P, 4 * q], dt, tag="r1")
                nc.vector.tensor_scalar(out=r1[:], in0=dn[:], scalar1=0.5,
                                        scalar2=0.0, op0=OP.subtract, op1=OP.max)
                jR = dn if packed else jA
                nc.scalar.activation(out=jR[:], in_=dn[:], func=AF.Relu,
                                     bias=negh[:, 0:1], scale=-1.0,
                                     accum_out=s_r2[:, i:i + 1])
                if 'pe' not in skip:
                    na = 4 * q // P
                    for c in range(na):
                        sl = slice(c * P, (c + 1) * P)
                        st, sp = (first and c == 0), (last and c == na - 1)
                        nc.tensor.matmul(out=pR1[:], lhsT=r1[:, sl],
                                         rhs=ones1[:, 0:1], start=st, stop=sp)
                        nc.tensor.matmul(out=pDN[:], lhsT=dn[:, sl],
                                         rhs=dn[:, sl], start=st, stop=sp)

                # ---------- scalar (PE only) ----------
                if 'pe' not in skip:
                    ns_ = 2 * q // P
                    for c in range(ns_):
                        sl = slice(c * P, (c + 1) * P)
                        st, sp = (first and c == 0), (last and c == ns_ - 1)
                        nc.tensor.matmul(out=pOO[:], lhsT=o_sc[:, sl],
                                         rhs=o_sc[:, sl], start=st, stop=sp)
                        nc.tensor.matmul(out=pOT[:], lhsT=o_sc[:, sl],
                                         rhs=t_sc[:, sl], start=st, stop=sp)
                        nc.tensor.matmul(out=pTT[:], lhsT=t_sc[:, sl],
                                         rhs=t_sc[:, sl], start=st, stop=sp)

                # ---------- vec ----------
                sq = scr.tile([P, 3 * q], dt, tag="sq")
                nc.scalar.activation(out=sq[:], in_=o_v, func=AF.Square)
                n2 = scr.tile([P, q], dt, tag="n2")
                geng = nc.vector
                geng.tensor_tensor(out=n2[:], in0=sq[:, 0:q],
                                   in1=sq[:, q:2 * q], op=OP.add)
                geng.tensor_tensor(out=n2[:], in0=n2[:],
                                   in1=sq[:, 2 * q:3 * q], op=OP.add)
                nc.scalar.activation(out=n2[:], in_=n2[:], func=AF.Sqrt,
                                     accum_out=s_nrm[:, i:i + 1])
                nrm = n2
                w = scr.tile([P, 3 * q], dt, tag="w")
                nc.vector.tensor_scalar(out=w[:], in0=o_v, scalar1=0.0,
                                        scalar2=1.0, op0=OP.is_lt, op1=OP.mult)
                for b in range(3):
                    geng.tensor_tensor(out=w[:, b * q:(b + 1) * q],
                                       in0=w[:, b * q:(b + 1) * q],
                                       in1=nrm[:], op=OP.mult)
                wn = w
                if packed:
                    a = ot[:, 5 * q:8 * q]
                    nc.vector.tensor_tensor(out=a, in0=o_v, in1=t_v,
                                            op=OP.subtract)
                else:
                    at = scr.tile([P, 3 * q], dt, tag="at")
                    nc.vector.tensor_tensor(out=at[:], in0=o_v, in1=t_v,
                                            op=OP.subtract)
                    a = at[:]
                if 'pe' not in skip:
                    nv = 3 * q // P
                    for c in range(nv):
                        sl = slice(c * P, (c + 1) * P)
                        st, sp = (first and c == 0), (last and c == nv - 1)
                        nc.tensor.matmul(out=pAA[:], lhsT=a[:, sl], rhs=a[:, sl],
                                         start=st, stop=sp)
                        nc.tensor.matmul(out=pAW[:], lhsT=a[:, sl], rhs=wn[:, sl],
                                         start=st, stop=sp)
                        nc.tensor.matmul(out=pWW[:], lhsT=wn[:, sl], rhs=wn[:, sl],
                                         start=st, stop=sp)

            # ---------- drain ----------
            gsb = acc.tile([P, NG * P + 1], f32, tag="gsb")
            if mode == 'dma' or 'pe' in skip:
                nc.vector.memset(gsb[:], 0.0)
            else:
                for j, g in enumerate([pOO, pOT, pTT, pAA, pAW, pWW, pDN]):
                    nc.vector.tensor_copy(gsb[:, j * P:(j + 1) * P], g[:])
                nc.vector.tensor_copy(gsb[:, NG * P:NG * P + 1], pR1[:])
            out_sb = acc.tile([P, 8], f32, tag="out_sb")
            nc.vector.memset(out_sb[:], 0.0)
            for j, s in enumerate(accs):
                nc.vector.tensor_reduce(out=out_sb[:, j:j + 1], in_=s[:],
                                        axis=mybir.AxisListType.X, op=OP.add)
            nc.sync.dma_start(out=out_d[:], in_=out_sb[:])
            nc.sync.dma_start(out=gram_d[:], in_=gsb[:])

    nc.compile()
    return nc


def _prep(arr, shard, core, q=Q, half=True):
    sl = arr[core * shard:(core + 1) * shard, :]
    n_tiles = shard // (P * q)
    a = sl.reshape(n_tiles, P, q, 9).transpose(0, 1, 3, 2)[:, :, PERM, :]
    out = np.ascontiguousarray(a, dtype=np.float16 if half else np.float32)
    return out.reshape(n_tiles * P, 9 * q)


def _finish(partials, grams, batch):
    tot = partials.astype(np.float64).sum(axis=(0, 1))
    ext, R2, nrm = tot[0], tot[1], tot[2]
    g = grams.astype(np.float64)
    tr = lambda j: np.trace(g[:, :, j * P:(j + 1) * P], axis1=1, axis2=2).sum()
    R1 = g[:, :, NG * P].sum()
    sdn2 = tr(6)
    n4 = 4.0 * batch
    p0 = tr(0) - 2.0 * tr(1) + tr(2)
    p2 = tr(3) + 2.0 * tr(4) + tr(5)
    # sum max(|dn|,.5) = R1 + R2 + .5 n4 ;  p1 = sdn2 - 2*smax + n4
    p1 = sdn2 - 2.0 * (R1 + R2) - n4 + n4
    c0 = ext / batch / CONSTANT_WEIGHT
    c1 = nrm / batch / CONSTANT_WEIGHT
    mse = (p0 + p1 + p2) / (batch * 9)
    if (p0 > p1) and (p0 > p2):
        amount = 0.0
    elif (p0 > p1) and (p0 < p2):
        amount = c1
    elif (p0 < p1) and (p0 > p2):
        amount = c0
    else:
        amount = c0 + c1
    return np.float32(mse + amount)


def _run(outputs, targets, shard, q, n_tiles, n_cores, half=HALF, **spmd_kwargs):
    key = (shard, q, n_tiles, half)
    if key not in _cache:
        _cache[key] = _build(shard, q, n_tiles, half=half)
    nc = _cache[key]
    in_maps = [{"o": _prep(outputs, shard, k, q, half),
                "t": _prep(targets, shard, k, q, half)}
               for k in range(n_cores)]
    br = run_bass_kernel_spmd(nc, in_maps, list(range(n_cores)), **spmd_kwargs)
    partials = np.stack([r["partials"] for r in br.results])
    grams = np.stack([r["gram"] for r in br.results])
    return partials, grams


def kernel(outputs, targets):
    outputs = np.asarray(outputs)
    targets = np.asarray(targets)
    assert outputs.shape == (BATCH, 9), outputs.shape
    partials, grams = _run(outputs, targets, SHARD, Q, N_TILES, N_CORES)
    return _finish(partials, grams, BATCH)
